# revision 8
# baseline (speedup 1.0000x reference)
"""Trainium2 Bass kernel for CombinedLossExp72 (feature MSE + triplet + InfoNCE
with hard-negative mining over a 4096x512 codebook).

Strategy (data-parallel over the batch axis, 8 cores x 2048 tokens):
  per 128-token tile:
    mm1 (PE):  W = x @ cb^T - c2/2           (bf16 inputs, exact-f32 column bias
                                              via an extra f32r rank-1 matmul)
    mm2 (PE):  Gn = x @ normalize(cb)^T      (bf16)
    ACT:       E = exp(Gn * 1/(T*||x||))     straight out of PSUM
    DVE:       spike the positive code, top-16 selection with
               max8 -> match_replace -> max8 -> match_replace (sentinel),
               then one fused (W==sentinel)*E pass with accum_out giving
               sum_{hard negs} exp(sim/T).
    losses:    feature/triplet/positive-logit ride along on ACT/Pool.
  Host: shard, transpose, bf16 casts, codebook stats, positive gather,
        final scalar combine.
"""

import numpy as np
import ml_dtypes
from contextlib import ExitStack

B, T, D, K = 8, 2048, 512, 4096
NCORES = 8
TOK = (B * T) // NCORES      # tokens per core
P = 128
NTILES = TOK // P            # 16
NKBLK = K // 512             # 8 psum bank blocks
NCHUNK = D // P              # 4 contraction chunks
MARGIN, TEMP = 0.2, 0.1
FEATURE_W, TRIPLET_W, CONTRASTIVE_W = 1.0, 1.0, 0.5
SPIKE = -1.0e30              # added at the positive code's column
SENT = -2.0e30               # match_replace sentinel marking selected negatives


def emit(tc, ins, outs, ntiles=NTILES):
    import concourse.bass as bass  # noqa: F401
    from concourse import mybir

    nc = tc.nc
    f32 = mybir.dt.float32
    bf16 = mybir.dt.bfloat16
    f32r = mybir.dt.float32r
    AF = mybir.ActivationFunctionType
    OP = mybir.AluOpType
    AX = mybir.AxisListType.X

    x_nat = ins["x_nat"]
    xT_bf = ins["xT_bf"]
    t_nat = ins["t_nat"]
    tn_nat = ins["tn_nat"]
    p_nat = ins["pos_nat"]
    cb_bf = ins["cb_bf"]
    cn_bf = ins["cn_bf"]
    c2h = ins["c2h_neg"]
    codes_f = ins["codes_f"]
    out_part = outs["out_part"]

    with ExitStack() as ctx:
        const = ctx.enter_context(tc.tile_pool(name="const", bufs=1))
        iop = ctx.enter_context(tc.tile_pool(name="io", bufs=2))
        work = ctx.enter_context(tc.tile_pool(name="work", bufs=1))
        sm = ctx.enter_context(tc.tile_pool(name="sm", bufs=4))
        colsp = ctx.enter_context(tc.tile_pool(name="cols", bufs=1))
        scr = ctx.enter_context(tc.tile_pool(name="scr", bufs=2))
        psum = ctx.enter_context(tc.tile_pool(name="psum", bufs=1, space="PSUM"))

        # ---- constants (loaded once) ----
        cb_c, cn_c = [], []
        for c in range(NCHUNK):
            cbt = const.tile([P, K], bf16, name=f"cb{c}")
            nc.sync.dma_start(cbt[:], cb_bf[c * P:(c + 1) * P, :])
            cb_c.append(cbt)
            cnt_ = const.tile([P, K], bf16, name=f"cn{c}")
            nc.sync.dma_start(cnt_[:], cn_bf[c * P:(c + 1) * P, :])
            cn_c.append(cnt_)
        c2h_sb = const.tile([1, K], f32r, name="c2h_sb")
        nc.sync.dma_start(c2h_sb[:], c2h[:])
        ones_sb = const.tile([1, P], f32r, name="ones_sb")
        nc.sync.dma_start(ones_sb[:], ins["ones_in"][:])
        iota_sb = const.tile([P, K], f32, name="iota_sb")
        nc.gpsimd.iota(iota_sb[:], pattern=[[1, K]], base=0, channel_multiplier=0,
                       allow_small_or_imprecise_dtypes=True)
        codes_sb = const.tile([P, NTILES], f32, name="codes_sb")
        nc.sync.dma_start(codes_sb[:], codes_f[:])
        margin_sb = const.tile([P, 1], f32, name="margin_sb")
        nc.vector.memset(margin_sb[:], MARGIN)

        featcols = colsp.tile([P, ntiles], f32, name="featcols")
        tripcols = colsp.tile([P, ntiles], f32, name="tripcols")
        cecols = colsp.tile([P, ntiles], f32, name="cecols")

        for t in range(ntiles):
            rs = slice(t * P, (t + 1) * P)
            x_t = iop.tile([P, D], f32, tag="x_t")
            nc.sync.dma_start(x_t[:], x_nat[rs, :])
            t_t = iop.tile([P, D], f32, tag="t_t")
            nc.sync.dma_start(t_t[:], t_nat[rs, :])
            tn_t = iop.tile([P, D], f32, tag="tn_t")
            nc.sync.dma_start(tn_t[:], tn_nat[rs, :])
            p_t = iop.tile([P, D], f32, tag="p_t")
            nc.sync.dma_start(p_t[:], p_nat[rs, :])
            xT_t = iop.tile([P, NCHUNK, P], bf16, tag="xT_t")
            for c in range(NCHUNK):
                nc.sync.dma_start(xT_t[:, c, :], xT_bf[c * P:(c + 1) * P, rs])

            # ---- per-token norms ----
            s0 = scr.tile([P, D], f32, tag="scr512")
            x2 = sm.tile([P, 1], f32, tag="x2")
            nc.scalar.activation(s0[:], x_t[:], AF.Square, accum_out=x2[:])
            sxT = sm.tile([P, 1], f32, tag="sxT")
            nc.scalar.activation(sxT[:], x2[:], AF.Sqrt, scale=TEMP * TEMP)
            rxoT = sm.tile([P, 1], f32, tag="rxoT")       # 1 / (T * ||x||)
            nc.vector.reciprocal(rxoT[:], sxT[:])

            s1 = scr.tile([P, D], f32, tag="scr512")
            p2 = sm.tile([P, 1], f32, tag="p2")
            nc.scalar.activation(s1[:], p_t[:], AF.Square, accum_out=p2[:])
            sp_ = sm.tile([P, 1], f32, tag="sp_")
            nc.scalar.activation(sp_[:], p2[:], AF.Sqrt)
            rp = sm.tile([P, 1], f32, tag="rp")           # 1 / ||pos||
            nc.vector.reciprocal(rp[:], sp_[:])

            # ---- feature + triplet ----
            dsc = scr.tile([P, D], f32, tag="dsc")
            nc.gpsimd.tensor_tensor(dsc[:], x_t[:], t_t[:], OP.subtract)
            s2 = scr.tile([P, D], f32, tag="scr512")
            nc.scalar.activation(s2[:], dsc[:], AF.Square,
                                 accum_out=featcols[:, t:t + 1])
            pd = sm.tile([P, 1], f32, tag="pd")
            nc.scalar.activation(pd[:], featcols[:, t:t + 1], AF.Sqrt)

            nsc = scr.tile([P, D], f32, tag="dsc")
            nc.gpsimd.tensor_tensor(nsc[:], x_t[:], tn_t[:], OP.subtract)
            nd2 = sm.tile([P, 1], f32, tag="nd2")
            s3 = scr.tile([P, D], f32, tag="scr512")
            nc.scalar.activation(s3[:], nsc[:], AF.Square, accum_out=nd2[:])
            ndist = sm.tile([P, 1], f32, tag="ndist")
            nc.scalar.activation(ndist[:], nd2[:], AF.Sqrt)
            tv = sm.tile([P, 1], f32, tag="tv")
            nc.vector.tensor_tensor(tv[:], pd[:], ndist[:], OP.subtract)
            nc.scalar.activation(tripcols[:, t:t + 1], tv[:], AF.Relu,
                                 bias=margin_sb[:])

            # ---- positive logit ----
            s4 = scr.tile([P, D], f32, tag="scr512")
            posdot = sm.tile([P, 1], f32, tag="posdot")
            nc.vector.scalar_tensor_tensor(s4[:], x_t[:], 0.0, p_t[:],
                                           OP.bypass, OP.mult, accum_out=posdot[:])
            l0 = sm.tile([P, 1], f32, tag="l0")
            nc.vector.tensor_scalar(l0[:], posdot[:], rxoT[:], rp[:], OP.mult, OP.mult)
            posexp = sm.tile([P, 1], f32, tag="posexp")
            nc.scalar.activation(posexp[:], l0[:], AF.Exp)

            # ---- mm1: W = x @ cb^T - c2/2 ----
            pg = psum.tile([P, K], f32, tag="psum", name="pg")
            for c in range(NCHUNK):
                for j in range(NKBLK):
                    nc.tensor.matmul(pg[:, j * 512:(j + 1) * 512],
                                     xT_t[:, c, :],
                                     cb_c[c][:, j * 512:(j + 1) * 512],
                                     start=(c == 0), stop=False)
            for j in range(NKBLK):
                nc.tensor.matmul(pg[:, j * 512:(j + 1) * 512],
                                 ones_sb[:],
                                 c2h_sb[:, j * 512:(j + 1) * 512],
                                 start=False, stop=True)

            # spike the positive code's column
            eqspike = work.tile([P, K], f32, tag="eqspike")
            nc.gpsimd.tensor_scalar(eqspike[:], iota_sb[:], codes_sb[:, t:t + 1],
                                    SPIKE, OP.is_equal, OP.mult)
            W = work.tile([P, K], f32, tag="W")
            nc.vector.scalar_tensor_tensor(W[:], pg[:], 0.0, eqspike[:],
                                           OP.bypass, OP.add)

            # ---- mm2: Gn = x @ cn^T ----
            pn = psum.tile([P, K], f32, tag="psum", name="pn")
            for c in range(NCHUNK):
                for j in range(NKBLK):
                    nc.tensor.matmul(pn[:, j * 512:(j + 1) * 512],
                                     xT_t[:, c, :],
                                     cn_c[c][:, j * 512:(j + 1) * 512],
                                     start=(c == 0), stop=(c == NCHUNK - 1))
            E = work.tile([P, K], f32, tag="E")
            nc.scalar.activation(E[:], pn[:], AF.Exp, scale=rxoT[:])

            # ---- top-16 selection on W ----
            m1 = sm.tile([P, 8], f32, tag="m1")
            nc.vector.max(m1[:], W[:])
            nc.vector.match_replace(W[:], m1[:], W[:], SENT)
            m2 = sm.tile([P, 8], f32, tag="m2")
            nc.vector.max(m2[:], W[:])
            nc.vector.match_replace(W[:], m2[:], W[:], SENT)

            # negsum = sum over selected of exp(sim/T)
            negsum = sm.tile([P, 1], f32, tag="negsum")
            nc.vector.scalar_tensor_tensor(E[:], W[:], SENT, E[:],
                                           OP.is_equal, OP.mult,
                                           accum_out=negsum[:])

            u = sm.tile([P, 1], f32, tag="u")
            nc.vector.tensor_tensor(u[:], negsum[:], posexp[:], OP.add)
            lse = sm.tile([P, 1], f32, tag="lse")
            nc.scalar.activation(lse[:], u[:], AF.Ln)
            nc.vector.tensor_tensor(cecols[:, t:t + 1], lse[:], l0[:], OP.subtract)

        outsb = colsp.tile([P, 4], f32, name="outsb")
        nc.vector.memset(outsb[:, 3:4], 0.0)
        nc.vector.tensor_reduce(outsb[:, 0:1], featcols[:], AX, OP.add)
        nc.vector.tensor_reduce(outsb[:, 1:2], tripcols[:], AX, OP.add)
        nc.vector.tensor_reduce(outsb[:, 2:3], cecols[:], AX, OP.add)
        nc.sync.dma_start(out_part[:], outsb[:])


def build(ntiles=NTILES):
    """Build + compile the Bacc program. Returns nc."""
    import concourse.bacc as bacc
    import concourse.tile as tile
    from concourse import mybir

    f32 = mybir.dt.float32
    bf16 = mybir.dt.bfloat16
    f32r = mybir.dt.float32r

    nc = bacc.Bacc("TRN2", target_bir_lowering=False, debug=False,
                   enable_asserts=False, num_devices=NCORES)
    ins = {
        "x_nat": nc.dram_tensor("x_nat", [TOK, D], f32, kind="ExternalInput").ap(),
        "xT_bf": nc.dram_tensor("xT_bf", [D, TOK], bf16, kind="ExternalInput").ap(),
        "t_nat": nc.dram_tensor("t_nat", [TOK, D], f32, kind="ExternalInput").ap(),
        "tn_nat": nc.dram_tensor("tn_nat", [TOK, D], f32, kind="ExternalInput").ap(),
        "pos_nat": nc.dram_tensor("pos_nat", [TOK, D], f32, kind="ExternalInput").ap(),
        "cb_bf": nc.dram_tensor("cb_bf", [D, K], bf16, kind="ExternalInput").ap(),
        "cn_bf": nc.dram_tensor("cn_bf", [D, K], bf16, kind="ExternalInput").ap(),
        "c2h_neg": nc.dram_tensor("c2h_neg", [1, K], f32r, kind="ExternalInput").ap(),
        "ones_in": nc.dram_tensor("ones_in", [1, P], f32r, kind="ExternalInput").ap(),
        "codes_f": nc.dram_tensor("codes_f", [P, NTILES], f32, kind="ExternalInput").ap(),
    }
    outs = {
        "out_part": nc.dram_tensor("out_part", [P, 4], f32, kind="ExternalOutput").ap(),
    }
    with tile.TileContext(nc) as tc:
        emit(tc, ins, outs, ntiles=ntiles)
    nc.compile()
    return nc


def make_in_maps(student_features, teacher_features, codebook, teacher_codes):
    """Host-side shard + layout prep. Returns list of 8 per-core input dicts."""
    x = np.ascontiguousarray(np.asarray(student_features, dtype=np.float32)).reshape(B * T, D)
    tch = np.ascontiguousarray(np.asarray(teacher_features, dtype=np.float32)).reshape(B, T, D)
    cb = np.ascontiguousarray(np.asarray(codebook, dtype=np.float32))
    codes = np.asarray(teacher_codes).reshape(B * T).astype(np.int64)

    c2 = (cb.astype(np.float64) ** 2).sum(axis=1)
    cnorm = np.sqrt(c2)
    cn = (cb / cnorm[:, None]).astype(np.float32)

    cb_bf = np.ascontiguousarray(cb.T).astype(ml_dtypes.bfloat16)
    cn_bf = np.ascontiguousarray(cn.T).astype(ml_dtypes.bfloat16)
    c2h_neg = (-0.5 * c2).astype(np.float32).reshape(1, K)

    in_maps = []
    for b in range(NCORES):
        sl = slice(b * TOK, (b + 1) * TOK)
        xs = x[sl]
        codes_s = codes[sl]
        in_maps.append({
            "x_nat": xs,
            "xT_bf": np.ascontiguousarray(xs.T).astype(ml_dtypes.bfloat16),
            "t_nat": np.ascontiguousarray(tch[b]),
            "tn_nat": np.ascontiguousarray(tch[(b - 1) % B]),
            "pos_nat": np.ascontiguousarray(cb[codes_s]),
            "cb_bf": cb_bf,
            "cn_bf": cn_bf,
            "c2h_neg": c2h_neg,
            "ones_in": np.ones((1, P), dtype=np.float32),
            "codes_f": np.ascontiguousarray(
                codes_s.reshape(NTILES, P).T).astype(np.float32),
        })
    return in_maps


def combine(results):
    """Combine per-core [128, 4] partials into the scalar loss."""
    feat = trip = ce = 0.0
    for r in results:
        p = np.asarray(r["out_part"], dtype=np.float64)
        feat += p[:, 0].sum()
        trip += p[:, 1].sum()
        ce += p[:, 2].sum()
    n = float(B * T)
    total = (FEATURE_W * feat / (n * D)
             + TRIPLET_W * trip / n
             + CONTRASTIVE_W * ce / n)
    return np.float32(total)


_NC_CACHE = None


def kernel(student_features, teacher_features, codebook, teacher_codes):
    global _NC_CACHE
    from concourse import bass_utils

    if _NC_CACHE is None:
        _NC_CACHE = build()
    nc = _NC_CACHE
    in_maps = make_in_maps(student_features, teacher_features, codebook,
                           teacher_codes)
    res = bass_utils.run_bass_kernel_spmd(nc, in_maps,
                                          core_ids=list(range(NCORES)))
    return combine(res.results)
